# revision 17
# baseline (speedup 1.0000x reference)
"""LogicMP mean-field message passing on 8 TRN2 NeuronCores.

Math (derived from the reference):
  softmax over 2 labels == sigmoid of the logit difference, so with
    ds0[i]   = smoke[i,1] - smoke[i,0]
    df[a,b]  = friend[a,b,1] - friend[a,b,0]          (fixed all iterations)
  each step t is
    s1^t = sigmoid(ds^t); s0^t = 1 - s1^t
    f1^t = sigmoid(df - w0 * outer(s1^{t-1}, s0^{t-1}))   (t=0: sigmoid(df))
    m_s0[a] = sum_b f1[a,b]*s0^t[b]                (local per row shard)
    m_s1[b] = sum_a s1^t[a]*f1[a,b]                (all-reduce over a)
    ds^{t+1} = ds0 + w0*(m_s1 - m_s0)
  outputs:
    out_s = smoke + w0*stack([m_s0, m_s1]) from the final step
    out_f = friend + stack([w0*outer(s1, s0), 0]) with the final s

Sharding: friend rows split 8 ways (512 rows/core); smoke + weight replicated.
df stays SBUF-resident per core (8 MB fp32); per-iteration cross-core traffic is
one 32 KB AllReduce of packed [m_s0 (own slot) | m_s1 partial].

Small [n] vectors are handled in "rows form" [CC, 128] (partition c holds
vec[128c:128c+128]) so every DRAM transfer of them is contiguous per
partition; fine-grained interleaved DRAM writes wedge the device.
"""

import numpy as np

N = 4096
NCORES = 8
NSTEP = 5

_CACHE = {}


def _build(n, ncores, est=False):
    from contextlib import ExitStack

    import concourse.bass as bass
    import concourse.bacc as bacc
    import concourse.tile as tile
    from concourse import mybir

    f32 = mybir.dt.float32
    Alu = mybir.AluOpType
    Act = mybir.ActivationFunctionType

    S = n // ncores          # rows per core
    KT = S // 128            # a-tiles per core
    CC = n // 128            # rows-form partitions
    H = n // 2               # half-row width
    NJ = H // 512            # 512-wide chunks per half

    nc = bacc.Bacc(trn_type="TRN2", num_devices=(1 if est else ncores))

    friend = nc.declare_dram_parameter("friend", [S, n, 2], f32, isOutput=False)
    smoke = nc.declare_dram_parameter("smoke", [n, 2], f32, isOutput=False)
    wtens = nc.declare_dram_parameter("w", [1, 1], f32, isOutput=False)
    out_f = nc.declare_dram_parameter("out_f", [S, n, 2], f32, isOutput=True)
    out_s = nc.declare_dram_parameter("out_s", [n, 2], f32, isOutput=True)

    ar_in = nc.dram_tensor("ar_in", [1, 2 * n], f32)
    svec = nc.dram_tensor("svec", [1, n], f32)    # s0 of the current step
    s1vec = nc.dram_tensor("s1vec", [1, n], f32)  # s1 of the current step
    ar_out = nc.dram_tensor("ar_out", [1, 2 * n], f32, addr_space="Shared")

    fr_flat = friend.rearrange("(k p) b t -> k p (b t)", p=128)
    of_flat = out_f.rearrange("(k p) b t -> k p (b t)", p=128)

    def dram_rows(handle, off, width):
        """[CC, width] AP over a flat DRAM vector: partition c = [off + c*width, +width)."""
        return bass.AP(tensor=handle.ap().tensor, offset=off, ap=[[width, CC], [1, width]])

    def dram_bcast(handle, off, width, parts=128):
        return bass.AP(tensor=handle.ap().tensor, offset=off, ap=[[0, parts], [1, width]])

    with ExitStack() as ctx:
        tc = ctx.enter_context(tile.TileContext(nc))
        singles = ctx.enter_context(tc.tile_pool(name="singles", bufs=1))
        vecs = ctx.enter_context(tc.tile_pool(name="vecs", bufs=2))
        work = ctx.enter_context(tc.tile_pool(name="work", bufs=2))
        stage = ctx.enter_context(tc.tile_pool(name="stage", bufs=3))
        pp = ctx.enter_context(tc.tile_pool(name="pp", bufs=1, space="PSUM"))

        pid = nc.partition_id()

        # ---------------- persistent tiles ----------------
        df = [singles.tile([128, n], f32, tag=f"df{k}", name=f"df{k}") for k in range(KT)]
        smoke_r = singles.tile([CC, 256], f32, tag="smoke_r")
        ds0r = singles.tile([CC, 128], f32, tag="ds0r")
        w0col = singles.tile([128, 1], f32, tag="w0col")
        ms1row = singles.tile([1, n], f32, tag="ms1row")
        s1own = singles.tile([128, KT], f32, tag="s1own")
        w1own_t = singles.tile([128, KT], f32, tag="w1own_t")

        # ---------------- prep ----------------
        nc.sync.dma_start(out=smoke_r, in_=dram_rows(smoke, 0, 256))
        nc.sync.dma_start(out=w0col, in_=dram_bcast(wtens, 0, 1))
        # zero the all-reduce input once: other cores' m_s0 slots stay zero
        nc.vector.memset(ms1row, 0.0)
        nc.sync.dma_start(out=ar_in[0:1, 0:n], in_=ms1row)
        nc.sync.dma_start(out=ar_in[0:1, n:2 * n], in_=ms1row)

        sm3 = smoke_r.rearrange("c (q t) -> c q t", t=2)
        nc.vector.tensor_tensor(ds0r, sm3[:, :, 1], sm3[:, :, 0], Alu.subtract)

        s1r = vecs.tile([CC, 128], f32, tag="s1r")
        s0r = vecs.tile([CC, 128], f32, tag="s0r")
        nc.scalar.activation(s1r, ds0r, Act.Sigmoid)
        nc.vector.tensor_scalar(s0r, s1r, -1.0, 1.0, Alu.mult, Alu.add)

        # friend load -> df ([128, H] chunks of the [b][t]-flat columns)
        for k in range(KT):
            for j in range(4):
                pt = stage.tile([128, H], f32, tag="stage")
                nc.sync.dma_start(out=pt, in_=fr_flat[k, :, j * H:(j + 1) * H])
                pt3 = pt.rearrange("p (b t) -> p b t", t=2)
                q = H // 2
                nc.vector.tensor_tensor(
                    df[k][:, j * q:(j + 1) * q], pt3[:, :, 1], pt3[:, :, 0],
                    Alu.subtract,
                )

        s0b_prev = None
        w1_prev = None
        nw1_prev = None

        # ---------------- iterations ----------------
        for t in range(NSTEP):
            # publish current s to DRAM (contiguous), derive broadcast + own forms
            nc.sync.dma_start(out=dram_rows(svec, 0, 128), in_=s0r)
            nc.sync.dma_start(out=dram_rows(s1vec, 0, 128), in_=s1r)
            s0b = vecs.tile([128, n], f32, tag="s0b")
            nc.sync.dma_start(out=s0b, in_=dram_bcast(svec, 0, n))
            # own-rows gather: s1own[q, k] = s1[pid*S + 128k + q]
            sl = s1vec[0:1, bass.ds(pid * S, S)]
            nc.gpsimd.dma_start(
                out=s1own,
                in_=bass.AP(tensor=sl.tensor, offset=sl.offset, ap=[[1, 128], [128, KT]]),
            )
            w1own = vecs.tile([128, KT], f32, tag="w1own")
            nc.vector.tensor_scalar(w1own, s1own, w0col[:, 0:1], None, Alu.mult)
            nw1 = vecs.tile([128, KT], f32, tag="nw1")
            nc.vector.tensor_scalar(nw1, w1own, -1.0, None, Alu.mult)

            acc = vecs.tile([128, KT], f32, tag="acc")
            acch = vecs.tile([128, KT], f32, tag="acch")
            ps = [pp.tile([1, 512], f32, tag=f"ps{j}", name=f"ps{j}_{t}") for j in range(2 * NJ)]

            for k in range(KT):
                for h in range(2):
                    dfs = df[k][:, h * H:(h + 1) * H]
                    o = work.tile([128, H], f32, tag="o")
                    if t > 0:
                        nc.vector.affine_then_add(
                            o, s0b_prev[:, h * H:(h + 1) * H], dfs,
                            scale=nw1_prev[:, k:k + 1], bias=0.0,
                        )
                        zin = o
                    else:
                        zin = dfs
                    f1 = work.tile([128, H], f32, tag="f1")
                    nc.scalar.activation(f1, zin, Act.Sigmoid)
                    # m_s0 partial (raw): fused mult+reduce, combine halves after
                    nc.vector.affine_mul_reduce(
                        out=o, in0=f1, in1=s0b[:, h * H:(h + 1) * H],
                        scale=1.0, bias=0.0,
                        accum_out=(acch[:, k:k + 1] if h == 0 else acc[:, k:k + 1]),
                    )
                    if h == 1:
                        nc.vector.tensor_tensor(
                            acc[:, k:k + 1], acc[:, k:k + 1], acch[:, k:k + 1], Alu.add
                        )
                    # m_s1 partial (raw)
                    for j in range(NJ):
                        nc.tensor.matmul(
                            ps[h * NJ + j][0:1, :],
                            s1own[:, k:k + 1],
                            f1[:, j * 512:(j + 1) * 512],
                            start=(k == 0), stop=(k == KT - 1),
                        )

            for j in range(2 * NJ):
                nc.scalar.activation(
                    ms1row[0:1, j * 512:(j + 1) * 512], ps[j][0:1, :], Act.Copy
                )

            # stage + all-reduce (raw sums; w0 applied after)
            slot = ar_in[0:1, bass.ds(pid * S, S)]
            slot_col = bass.AP(tensor=slot.tensor, offset=slot.offset, ap=[[1, 128], [128, KT]])
            nc.gpsimd.dma_start(out=slot_col, in_=acc)
            nc.sync.dma_start(out=ar_in[0:1, n:2 * n], in_=ms1row)
            if est:
                # cost-model stand-in for the AllReduce (same dataflow shape)
                nc.sync.dma_start(out=ar_out[0:1, :], in_=ar_in[0:1, :])
            else:
                nc.gpsimd.collective_compute(
                    "AllReduce", Alu.add,
                    replica_groups=[list(range(ncores))],
                    ins=[ar_in.ap()], outs=[ar_out.ap()],
                )

            ars0 = vecs.tile([CC, 128], f32, tag="ars0")
            ars1 = vecs.tile([CC, 128], f32, tag="ars1")
            nc.sync.dma_start(out=ars0, in_=dram_rows(ar_out, 0, 128))
            nc.sync.dma_start(out=ars1, in_=dram_rows(ar_out, n, 128))

            s0b_prev = s0b
            w1_prev = w1own
            nw1_prev = nw1

            if t < NSTEP - 1:
                dsn = vecs.tile([CC, 128], f32, tag="dsn")
                nc.vector.tensor_tensor(dsn, ars1, ars0, Alu.subtract)
                nc.vector.tensor_scalar(dsn, dsn, w0col[0:CC, 0:1], None, Alu.mult)
                nc.vector.tensor_tensor(dsn, ds0r, dsn, Alu.add)
                s1r = vecs.tile([CC, 128], f32, tag="s1r")
                s0r = vecs.tile([CC, 128], f32, tag="s0r")
                nc.scalar.activation(s1r, dsn, Act.Sigmoid)
                nc.vector.tensor_scalar(s0r, s1r, -1.0, 1.0, Alu.mult, Alu.add)
            else:
                # out_s = smoke + w0 * [m_s0 | m_s1]
                ost = vecs.tile([CC, 256], f32, tag="ost")
                ost3 = ost.rearrange("c (q t) -> c q t", t=2)
                for ch, ar in ((0, ars0), (1, ars1)):
                    nc.vector.tensor_scalar(ost3[:, :, ch], ar, w0col[0:CC, 0:1], None, Alu.mult)
                    nc.vector.tensor_tensor(ost3[:, :, ch], sm3[:, :, ch], ost3[:, :, ch], Alu.add)
                nc.sync.dma_start(out=dram_rows(out_s, 0, 256), in_=ost)

        # ---------------- out_f pass ----------------
        # s0b_prev / w1_prev hold the final-step s (s^{NSTEP-1}), exactly the
        # s used for the reference's final friend message.
        for k in range(KT):
            for h in range(2):
                o4 = work.tile([128, H], f32, tag="o")
                nc.vector.tensor_scalar(
                    o4, s0b_prev[:, h * H:(h + 1) * H], w1_prev[:, k:k + 1], None, Alu.mult
                )
                for j in range(2):
                    jj = 2 * h + j
                    pt = stage.tile([128, H], f32, tag="stage")
                    nc.sync.dma_start(out=pt, in_=fr_flat[k, :, jj * H:(jj + 1) * H])
                    pt3 = pt.rearrange("p (b t) -> p b t", t=2)
                    q = H // 2
                    nc.vector.tensor_tensor(
                        pt3[:, :, 0], pt3[:, :, 0], o4[:, j * q:(j + 1) * q], Alu.add
                    )
                    nc.sync.dma_start(out=of_flat[k, :, jj * H:(jj + 1) * H], in_=pt)

    nc.compile()
    return nc


def _get_nc(n=N, ncores=NCORES, est=False):
    key = (n, ncores, est)
    if key not in _CACHE:
        _CACHE[key] = _build(n, ncores, est=est)
    return _CACHE[key]


def kernel(logits_smoke, logits_friend, weights):
    from concourse.bass_utils import run_bass_kernel_spmd

    n = logits_smoke.shape[0]
    S = n // NCORES
    nc = _get_nc(n, NCORES)

    smoke = np.ascontiguousarray(np.asarray(logits_smoke, dtype=np.float32))
    friend = np.ascontiguousarray(np.asarray(logits_friend, dtype=np.float32))
    w = np.asarray(weights, dtype=np.float32).reshape(1, 1)

    in_maps = [
        {
            "friend": np.ascontiguousarray(friend[c * S:(c + 1) * S]),
            "smoke": smoke,
            "w": w,
        }
        for c in range(NCORES)
    ]
    res = run_bass_kernel_spmd(nc, in_maps, core_ids=list(range(NCORES)))
    outs = res.results
    out_f = np.concatenate(
        [np.asarray(outs[c]["out_f"]).reshape(S, n, 2) for c in range(NCORES)], axis=0
    )
    out_s = np.asarray(outs[0]["out_s"]).reshape(n, 2)
    return out_s, out_f


# revision 19
# speedup vs baseline: 1.1575x; 1.1575x over previous
"""LogicMP mean-field message passing on 8 TRN2 NeuronCores.

Math (derived from the reference):
  softmax over 2 labels == sigmoid of the logit difference, so with
    ds0[i]   = smoke[i,1] - smoke[i,0]
    df[a,b]  = friend[a,b,1] - friend[a,b,0]          (fixed all iterations)
  each step t is
    s1^t = sigmoid(ds^t); s0^t = 1 - s1^t
    f1^t = sigmoid(df - w0 * outer(s1^{t-1}, s0^{t-1}))   (t=0: sigmoid(df))
    m_s0[a] = sum_b f1[a,b]*s0^t[b]                (local per row shard)
    m_s1[b] = sum_a s1^t[a]*f1[a,b]                (all-reduce over a)
    ds^{t+1} = ds0 + w0*(m_s1 - m_s0)
  outputs:
    out_s = smoke + w0*stack([m_s0, m_s1]) from the final step
    out_f = friend + stack([w0*outer(s1, s0), 0]) with the final s

Sharding: friend rows split 8 ways (512 rows/core); smoke + weight replicated.
df stays SBUF-resident per core (8 MB fp32); per-iteration cross-core traffic is
one 32 KB AllReduce of packed [m_s0 (own slot) | m_s1 partial].

Small [n] vectors are handled in "rows form" [CC, 128] (partition c holds
vec[128c:128c+128]) so every DRAM transfer of them is contiguous per
partition; fine-grained interleaved DRAM writes wedge the device.
"""

import numpy as np

N = 4096
NCORES = 8
NSTEP = 5

_CACHE = {}


def _build(n, ncores, est=False):
    from contextlib import ExitStack

    import concourse.bass as bass
    import concourse.bacc as bacc
    import concourse.tile as tile
    from concourse import mybir

    f32 = mybir.dt.float32
    Alu = mybir.AluOpType
    Act = mybir.ActivationFunctionType

    S = n // ncores          # rows per core
    KT = S // 128            # a-tiles per core
    CC = n // 128            # rows-form partitions
    H = n // 2               # half-row width
    NJ = H // 512            # 512-wide chunks per half

    nc = bacc.Bacc(trn_type="TRN2", num_devices=(1 if est else ncores))

    friend = nc.declare_dram_parameter("friend", [S, n, 2], f32, isOutput=False)
    smoke = nc.declare_dram_parameter("smoke", [n, 2], f32, isOutput=False)
    wtens = nc.declare_dram_parameter("w", [1, 1], f32, isOutput=False)
    out_f = nc.declare_dram_parameter("out_f", [S, n, 2], f32, isOutput=True)
    out_s = nc.declare_dram_parameter("out_s", [n, 2], f32, isOutput=True)

    ar_in = nc.dram_tensor("ar_in", [1, 2 * n], f32)
    svec = nc.dram_tensor("svec", [1, n], f32)    # s0 of the current step
    s1vec = nc.dram_tensor("s1vec", [1, n], f32)  # s1 of the current step
    ar_out = nc.dram_tensor("ar_out", [1, 2 * n], f32, addr_space="Shared")

    fr_flat = friend.rearrange("(k p) b t -> k p (b t)", p=128)
    of_flat = out_f.rearrange("(k p) b t -> k p (b t)", p=128)

    def dram_rows(handle, off, width):
        """[CC, width] AP over a flat DRAM vector: partition c = [off + c*width, +width)."""
        return bass.AP(tensor=handle.ap().tensor, offset=off, ap=[[width, CC], [1, width]])

    def dram_bcast(handle, off, width, parts=128):
        return bass.AP(tensor=handle.ap().tensor, offset=off, ap=[[0, parts], [1, width]])

    with ExitStack() as ctx:
        tc = ctx.enter_context(tile.TileContext(nc))
        singles = ctx.enter_context(tc.tile_pool(name="singles", bufs=1))
        vecs = ctx.enter_context(tc.tile_pool(name="vecs", bufs=2))
        work = ctx.enter_context(tc.tile_pool(name="work", bufs=2))
        stage = ctx.enter_context(tc.tile_pool(name="stage", bufs=3))
        pp = ctx.enter_context(tc.tile_pool(name="pp", bufs=1, space="PSUM"))

        pid = nc.partition_id()

        # ---------------- persistent tiles ----------------
        df = [singles.tile([128, n], f32, tag=f"df{k}", name=f"df{k}") for k in range(KT)]
        smoke_r = singles.tile([CC, 256], f32, tag="smoke_r")
        ds0r = singles.tile([CC, 128], f32, tag="ds0r")
        w0col = singles.tile([128, 1], f32, tag="w0col")
        ms1row = singles.tile([1, n], f32, tag="ms1row")
        s1own = singles.tile([128, KT], f32, tag="s1own")
        w1own_t = singles.tile([128, KT], f32, tag="w1own_t")

        # ---------------- prep ----------------
        nc.sync.dma_start(out=smoke_r, in_=dram_rows(smoke, 0, 256))
        nc.sync.dma_start(out=w0col, in_=dram_bcast(wtens, 0, 1))
        # zero the all-reduce input once: other cores' m_s0 slots stay zero
        nc.vector.memset(ms1row, 0.0)
        nc.sync.dma_start(out=ar_in[0:1, 0:n], in_=ms1row)
        nc.sync.dma_start(out=ar_in[0:1, n:2 * n], in_=ms1row)

        sm3 = smoke_r.rearrange("c (q t) -> c q t", t=2)
        nc.vector.tensor_tensor(ds0r, sm3[:, :, 1], sm3[:, :, 0], Alu.subtract)

        s1r = vecs.tile([CC, 128], f32, tag="s1r")
        s0r = vecs.tile([CC, 128], f32, tag="s0r")
        nc.scalar.activation(s1r, ds0r, Act.Sigmoid)
        nc.vector.tensor_scalar(s0r, s1r, -1.0, 1.0, Alu.mult, Alu.add)

        # friend load -> df ([128, H] chunks of the [b][t]-flat columns)
        for k in range(KT):
            for j in range(4):
                pt = stage.tile([128, H], f32, tag="stage")
                nc.sync.dma_start(out=pt, in_=fr_flat[k, :, j * H:(j + 1) * H])
                pt3 = pt.rearrange("p (b t) -> p b t", t=2)
                q = H // 2
                nc.vector.tensor_tensor(
                    df[k][:, j * q:(j + 1) * q], pt3[:, :, 1], pt3[:, :, 0],
                    Alu.subtract,
                )

        s0b_prev = None
        w1_prev = None
        nw1_prev = None

        # ---------------- iterations ----------------
        for t in range(NSTEP):
            # publish current s to DRAM (contiguous), derive broadcast + own forms
            nc.sync.dma_start(out=dram_rows(svec, 0, 128), in_=s0r)
            nc.sync.dma_start(out=dram_rows(s1vec, 0, 128), in_=s1r)
            s0b = vecs.tile([128, n], f32, tag="s0b")
            nc.sync.dma_start(out=s0b, in_=dram_bcast(svec, 0, n))
            # own-rows gather: s1own[q, k] = s1[pid*S + 128k + q]
            sl = s1vec[0:1, bass.ds(pid * S, S)]
            nc.gpsimd.dma_start(
                out=s1own,
                in_=bass.AP(tensor=sl.tensor, offset=sl.offset, ap=[[1, 128], [128, KT]]),
            )
            s1own_r = vecs.tile([128, KT], mybir.dt.float32r, tag="s1own_r")
            nc.vector.tensor_copy(s1own_r, s1own)
            w1own = vecs.tile([128, KT], f32, tag="w1own")
            nc.vector.tensor_scalar(w1own, s1own, w0col[:, 0:1], None, Alu.mult)
            nw1 = vecs.tile([128, KT], f32, tag="nw1")
            nc.vector.tensor_scalar(nw1, w1own, -1.0, None, Alu.mult)

            acc = vecs.tile([128, KT], f32, tag="acc")
            acch = vecs.tile([128, KT], f32, tag="acch")
            ps = [pp.tile([1, 512], f32, tag=f"ps{j}", name=f"ps{j}_{t}") for j in range(2 * NJ)]

            for k in range(KT):
                for h in range(2):
                    dfs = df[k][:, h * H:(h + 1) * H]
                    o = work.tile([128, H], f32, tag="o")
                    if t > 0:
                        nc.vector.affine_then_add(
                            o, s0b_prev[:, h * H:(h + 1) * H], dfs,
                            scale=nw1_prev[:, k:k + 1], bias=0.0,
                        )
                        zin = o
                    else:
                        zin = dfs
                    f1 = work.tile([128, H], mybir.dt.float32r, tag="f1")
                    nc.scalar.activation(f1, zin, Act.Sigmoid)
                    # m_s0 partial (raw): fused mult+reduce, combine halves after
                    nc.vector.affine_mul_reduce(
                        out=o, in0=f1.bitcast(f32), in1=s0b[:, h * H:(h + 1) * H],
                        scale=1.0, bias=0.0,
                        accum_out=(acch[:, k:k + 1] if h == 0 else acc[:, k:k + 1]),
                    )
                    if h == 1:
                        nc.vector.tensor_tensor(
                            acc[:, k:k + 1], acc[:, k:k + 1], acch[:, k:k + 1], Alu.add
                        )
                    # m_s1 partial (raw)
                    for j in range(NJ):
                        nc.tensor.matmul(
                            ps[h * NJ + j][0:1, :],
                            s1own_r[:, k:k + 1],
                            f1[:, j * 512:(j + 1) * 512],
                            start=(k == 0), stop=(k == KT - 1),
                        )

            for j in range(2 * NJ):
                nc.scalar.activation(
                    ms1row[0:1, j * 512:(j + 1) * 512], ps[j][0:1, :], Act.Copy
                )

            # stage + all-reduce (raw sums; w0 applied after)
            slot = ar_in[0:1, bass.ds(pid * S, S)]
            slot_col = bass.AP(tensor=slot.tensor, offset=slot.offset, ap=[[1, 128], [128, KT]])
            nc.gpsimd.dma_start(out=slot_col, in_=acc)
            nc.sync.dma_start(out=ar_in[0:1, n:2 * n], in_=ms1row)
            if est:
                # cost-model stand-in for the AllReduce (same dataflow shape)
                nc.sync.dma_start(out=ar_out[0:1, :], in_=ar_in[0:1, :])
            else:
                nc.gpsimd.collective_compute(
                    "AllReduce", Alu.add,
                    replica_groups=[list(range(ncores))],
                    ins=[ar_in.ap()], outs=[ar_out.ap()],
                )

            ars0 = vecs.tile([CC, 128], f32, tag="ars0")
            ars1 = vecs.tile([CC, 128], f32, tag="ars1")
            nc.sync.dma_start(out=ars0, in_=dram_rows(ar_out, 0, 128))
            nc.sync.dma_start(out=ars1, in_=dram_rows(ar_out, n, 128))

            s0b_prev = s0b
            w1_prev = w1own
            nw1_prev = nw1

            if t < NSTEP - 1:
                dsn = vecs.tile([CC, 128], f32, tag="dsn")
                nc.vector.tensor_tensor(dsn, ars1, ars0, Alu.subtract)
                nc.vector.tensor_scalar(dsn, dsn, w0col[0:CC, 0:1], None, Alu.mult)
                nc.vector.tensor_tensor(dsn, ds0r, dsn, Alu.add)
                s1r = vecs.tile([CC, 128], f32, tag="s1r")
                s0r = vecs.tile([CC, 128], f32, tag="s0r")
                nc.scalar.activation(s1r, dsn, Act.Sigmoid)
                nc.vector.tensor_scalar(s0r, s1r, -1.0, 1.0, Alu.mult, Alu.add)
            else:
                # out_s = smoke + w0 * [m_s0 | m_s1]
                ost = vecs.tile([CC, 256], f32, tag="ost")
                ost3 = ost.rearrange("c (q t) -> c q t", t=2)
                for ch, ar in ((0, ars0), (1, ars1)):
                    nc.vector.tensor_scalar(ost3[:, :, ch], ar, w0col[0:CC, 0:1], None, Alu.mult)
                    nc.vector.tensor_tensor(ost3[:, :, ch], sm3[:, :, ch], ost3[:, :, ch], Alu.add)
                nc.sync.dma_start(out=dram_rows(out_s, 0, 256), in_=ost)

        # ---------------- out_f pass ----------------
        # s0b_prev / w1_prev hold the final-step s (s^{NSTEP-1}), exactly the
        # s used for the reference's final friend message.
        for k in range(KT):
            for h in range(2):
                o4 = work.tile([128, H], f32, tag="o")
                nc.vector.tensor_scalar(
                    o4, s0b_prev[:, h * H:(h + 1) * H], w1_prev[:, k:k + 1], None, Alu.mult
                )
                for j in range(2):
                    jj = 2 * h + j
                    pt = stage.tile([128, H], f32, tag="stage")
                    nc.sync.dma_start(out=pt, in_=fr_flat[k, :, jj * H:(jj + 1) * H])
                    pt3 = pt.rearrange("p (b t) -> p b t", t=2)
                    q = H // 2
                    nc.vector.tensor_tensor(
                        pt3[:, :, 0], pt3[:, :, 0], o4[:, j * q:(j + 1) * q], Alu.add
                    )
                    nc.sync.dma_start(out=of_flat[k, :, jj * H:(jj + 1) * H], in_=pt)

    nc.compile()
    return nc


def _get_nc(n=N, ncores=NCORES, est=False):
    key = (n, ncores, est)
    if key not in _CACHE:
        _CACHE[key] = _build(n, ncores, est=est)
    return _CACHE[key]


def kernel(logits_smoke, logits_friend, weights):
    from concourse.bass_utils import run_bass_kernel_spmd

    n = logits_smoke.shape[0]
    S = n // NCORES
    nc = _get_nc(n, NCORES)

    smoke = np.ascontiguousarray(np.asarray(logits_smoke, dtype=np.float32))
    friend = np.ascontiguousarray(np.asarray(logits_friend, dtype=np.float32))
    w = np.asarray(weights, dtype=np.float32).reshape(1, 1)

    in_maps = [
        {
            "friend": np.ascontiguousarray(friend[c * S:(c + 1) * S]),
            "smoke": smoke,
            "w": w,
        }
        for c in range(NCORES)
    ]
    res = run_bass_kernel_spmd(nc, in_maps, core_ids=list(range(NCORES)))
    outs = res.results
    out_f = np.concatenate(
        [np.asarray(outs[c]["out_f"]).reshape(S, n, 2) for c in range(NCORES)], axis=0
    )
    out_s = np.asarray(outs[0]["out_s"]).reshape(n, 2)
    return out_s, out_f
